# revision 1
# baseline (speedup 1.0000x reference)
"""ClusterLoss (mean-entropy + batch-entropy) Bass kernel for 8 trn2 cores.

Problem: block_feats [T=4096, M*K=64*256] f32.
  x = reshape(T, M, K)
  L1 = mean over (T, M) of entropy(softmax(x, axis=K))
  L2 = -sum_m entropy(softmax(mean_t x)) / M
  out = L1 + L2   (scalar)

Sharding: columns across 8 cores (each core: 8 blocks x all 4096 rows).
 - Per-(row, block) entropies are core-local -> scalar partial sum.
 - Per-block batch-means are core-local (full T on-core)   -> scalar partial.
 - Single AllReduce of [1, 2] f32 combines the partials; every core emits the
   same final scalar.

Per-core engine plan (normal row-major layout, rows on partitions):
 - DMA  : 8 super-tiles [128, 4*2048] (4 row-groups packed in free dim).
 - ACT  : e = exp(x) one instruction per super-tile.
 - DVE  : per 256-segment s = sum(e)      via tensor_scalar  + accum_out
          per 256-segment u = sum(x*e)    via scalar_tensor_tensor + accum_out
 - PE   : column sums (for block means) via ones-matmul into PSUM,
          accumulated across all row groups; also final partition reduce.
 - tail : ent = ln(s) - u/s on [128, 256]; block-mean entropies from the
          colsum PSUM row; AllReduce add of [1,2]; final scalar.

Entropy is computed without the max-subtraction: inputs are N(0,1) (|x| < ~6),
exp() is safe in f32 and the result matches the stable reference to ~1e-6.
"""

import sys

sys.path.insert(0, "/opt/trn_rl_repo")

import numpy as np

import concourse.bass as bass
import concourse.bacc as bacc
import concourse.tile as tile
from concourse import mybir
from concourse.bass_utils import run_bass_kernel_spmd

F32 = mybir.dt.float32
BF16 = mybir.dt.bfloat16
AF = mybir.ActivationFunctionType
OP = mybir.AluOpType

# Problem constants
T = 4096            # rows (batch)
M_TOT = 64          # blocks
K = 256             # features per block
N_CORES = 8
COLS = (M_TOT * K) // N_CORES   # 2048 columns per core
M_LOC = COLS // K               # 8 blocks per core
P = 128                         # partitions
A = 4                           # row-groups packed per super-tile
ROWS_PER_TILE = P * A           # 512
NT = T // ROWS_PER_TILE         # 8 super-tiles

LMBDA = 1.0

# knobs
# x stays f32 (HWDGE loads, no Q7 descriptor-gen cost); e is bf16 (free cast
# out of the exp activation; gives the s-accumulate the 4x DVE mode).
GP_SEGS = 0          # Pool cannot run TS/STT (walrus engine check)
ACT_SEGS = 10        # per super-tile: s-segments (of 32) offloaded to ACT
BF16_TILES = tuple(range(NT))  # super-tiles loaded via SWDGE f32->bf16 cast
BUFS = 4             # rotation depth for streaming pools
USE_COLLECTIVE = True  # on-device AllReduce of the two partial scalars


def _absorb_deps(eng, dst_col, dep_insts):
    """Absorb cross-engine waits on `eng`'s queue before a wait-slot-limited
    instruction (e.g. SWDGE pseudo-DMA): one tiny input-free write per
    dependency, each carrying a single sem wait, advancing the engine's
    observed vector clock."""
    from concourse.tile_rust import add_dep_helper

    for j, di in enumerate(dep_insts):
        if hasattr(eng, "memset"):
            c = eng.memset(dst_col[:, j:j + 1], 0.0)
        else:
            c = eng.memzero(dst_col[:, j:j + 1])  # ScalarE
        add_dep_helper(c.ins, di.ins, reason="absorb wait for slot-limited op")


def _absorb(eng, dst_col, src_aps):
    """Absorb cross-engine waits: tiny copies that read the freshly produced
    tiles. Each copy carries one sem wait; once the engine has waited, its
    observed vector clock covers the tick, so the following TS/STT
    instructions (whose ISA structs carry only ONE sync wait slot) need no
    cross-engine waits. dst_col slices must be disjoint across calls to avoid
    same-engine WAW sem chains."""
    for j, src in enumerate(src_aps):
        eng.tensor_copy(dst_col[:, j:j + 1], src)


def build_nc(reps: int = 1):
    nc = bacc.Bacc("TRN2", target_bir_lowering=False, debug=False,
                   num_devices=N_CORES)
    x_dram = nc.dram_tensor("x", [T, COLS], F32, kind="ExternalInput")
    out_dram = nc.dram_tensor("out", [1, 1], F32, kind="ExternalOutput")

    edt = BF16

    from contextlib import ExitStack

    with tile.TileContext(nc) as tc, ExitStack() as ctx:
        loads = ctx.enter_context(tc.tile_pool(name="loads", bufs=BUFS))
        es = ctx.enter_context(tc.tile_pool(name="es", bufs=BUFS))
        junks = ctx.enter_context(tc.tile_pool(name="junks", bufs=2))
        junku = ctx.enter_context(tc.tile_pool(name="junku", bufs=2))
        singles = ctx.enter_context(tc.tile_pool(name="singles", bufs=1))
        psum = ctx.enter_context(tc.tile_pool(name="psum", bufs=1, space="PSUM"))
        dram = ctx.enter_context(tc.tile_pool(name="dram", bufs=1, space="DRAM"))

        if True:
            # persistent tiles
            ones_w = singles.tile([P, 1], F32, tag="ones_w")      # matmul lhsT
            nc.vector.memset(ones_w, 1.0)
            ones_b = singles.tile([P, 1], BF16, tag="ones_b")
            nc.vector.memset(ones_b, 1.0)
            s_sb = singles.tile([P, NT * A * M_LOC], F32, tag="s_sb")
            u_sb = singles.tile([P, NT * A * M_LOC], F32, tag="u_sb")
            # wait-absorber target (disjoint columns per use; see _absorb)
            GNT = reps * NT
            ab_v = singles.tile([P, 2 * GNT + reps], F32, tag="ab_v")
            ab_g = singles.tile([P, 2 * GNT], F32, tag="ab_g")
            ab_dma = singles.tile([P, 4 * GNT], F32, tag="ab_dma")
            ab_act = singles.tile([P, 2 * GNT], F32, tag="ab_act")
            ab_act2 = singles.tile([P, GNT], F32, tag="ab_act2")
            ab_t = singles.tile([1, 4 * reps], F32, tag="ab_t")
            ones_f32 = singles.tile([P, 1], F32, tag="ones_f32")
            nc.vector.memset(ones_f32, 1.0)

            # colsum accumulator in PSUM: [1, 2048] f32 (4 banks, partition 0)
            ps_cs = psum.tile([1, COLS], F32, tag="ps_cs")

            x_view = x_dram.ap().rearrange("(n a p) c -> n p a c", p=P, a=A)

            hist = {}  # git -> dict of instruction handles (buffer-reuse deps)
            for rep in range(reps):
              for it in range(NT):
                git = rep * NT + it
                if git >= 2:
                    # absorb WAR waits (readers of the recycled x_t/e_t slots)
                    # before the SWDGE DMA / ACT, whose ISA structs have too
                    # few sync-wait slots.
                    pv = hist[git - 2]
                    deps = [pv["act"], pv["stt"], pv["mm"]]
                    if "gstt" in pv:
                        deps.append(pv["gstt"])
                    _absorb_deps(nc.gpsimd,
                                 ab_dma[:, 4 * git:4 * git + len(deps)], deps)
                    _absorb_deps(nc.scalar, ab_act[:, 2 * git:2 * git + 1],
                                 [pv["stt"]])
                if rep > 0 and it == 0:
                    # new rep: ACT exp must also wait for the tail readers of
                    # the previous rep (cc path) and DMA; absorb on scalar q
                    pt = hist[git - 1]
                    _absorb_deps(nc.scalar, ab_act[:, 2 * git + 1:2 * git + 2],
                                 [pt["tail_dve"]])
                is_bf = it in BF16_TILES
                x_t = loads.tile([P, A, COLS], BF16 if is_bf else F32,
                                 tag="x_t")
                if is_bf:
                    dma_h = nc.gpsimd.dma_start(out=x_t[:], in_=x_view[it])
                else:
                    dma_h = nc.sync.dma_start(out=x_t[:], in_=x_view[it])

                e_t = es.tile([P, A, COLS], edt, tag="e_t")
                # absorb the DMA-done wait on the ACT queue (1-wait-slot limit)
                _absorb_deps(nc.scalar, ab_act2[:, git:git + 1], [dma_h])
                # e = exp(x); one big ACT op per super-tile
                act_h = nc.scalar.activation(e_t[:], x_t[:], AF.Exp)
                hist[git] = {"act": act_h, "dma": dma_h}

                # absorb ACT-done + DMA waits on the DVE / POOL queues
                _absorb(nc.vector, ab_v[:, 2 * git:2 * git + 2],
                        [x_t[:, 0, 0:1], e_t[:, 0, 0:1]])
                if GP_SEGS > 0:
                    _absorb(nc.gpsimd, ab_g[:, 2 * git:2 * git + 2],
                            [x_t[:, 0, 0:1], e_t[:, 0, 0:1]])

                # junk outputs for TS/STT: disjoint slices of rotating
                # tiles (a shared scratch creates same-engine WAW sem chains;
                # slices reused 8 segments apart only cost one wait slot)
                junk_s = junks.tile([P, COLS], edt, tag="junk_s")
                junk_u = junku.tile([P, COLS], edt, tag="junk_u")
                junk_g = junku.tile([P, COLS], edt, tag="junk_g")

                for a in range(A):
                    for m in range(M_LOC):
                        seg = a * M_LOC + m
                        idx = (it * A + a) * M_LOC + m
                        sl = (slice(None), a, slice(m * K, (m + 1) * K))
                        jl = (slice(None), slice(m * K, (m + 1) * K))
                        # s = sum_k e: single-src accumulate; ACT helps out
                        # (its input e_t is ACT-produced: no cross-engine dep)
                        if seg < ACT_SEGS:
                            nc.scalar.activation(
                                junk_s[jl], e_t[sl], AF.Copy,
                                accum_out=s_sb[:, idx:idx + 1])
                        else:
                            nc.vector.tensor_scalar(
                                out=junk_s[jl], in0=e_t[sl], scalar1=1.0,
                                scalar2=None, op0=OP.mult, op1=OP.add,
                                accum_out=s_sb[:, idx:idx + 1])
                        # u = sum_k x*e (fused mult + accum), split DVE/POOL
                        on_gp = (seg * GP_SEGS) % 32 < GP_SEGS  # interleaved
                        seg_engine = nc.gpsimd if on_gp else nc.vector
                        stt_h = seg_engine.scalar_tensor_tensor(
                            out=(junk_g if on_gp else junk_u)[jl],
                            in0=x_t[sl], scalar=1.0,
                            in1=e_t[sl], op0=OP.mult, op1=OP.mult,
                            accum_out=u_sb[:, idx:idx + 1])
                        hist[git]["gstt" if on_gp else "stt"] = stt_h

                # column sums for block means: ones^T @ x -> [1, COLS]
                for a in range(A):
                    for c in range(COLS // 512):
                        mm_h = nc.tensor.matmul(
                            ps_cs[0:1, c * 512:(c + 1) * 512],
                            ones_b[:] if is_bf else ones_w[:],
                            x_t[:, a, c * 512:(c + 1) * 512],
                            start=(it == 0 and a == 0),
                            stop=(it == NT - 1 and a == A - 1),
                        )
                        hist[git]["mm"] = mm_h

              # ---- tail: per-(row, block) entropies -> L1 partial ----
              n_col = NT * A * M_LOC  # 256
              ln_s = singles.tile([P, n_col], F32, tag="ln_s")
              nc.scalar.activation(ln_s[:], s_sb[:], AF.Ln)
              rs = singles.tile([P, n_col], F32, tag="rs")
              nc.vector.reciprocal(rs[:], s_sb[:])
              q = singles.tile([P, n_col], F32, tag="q")
              nc.vector.tensor_tensor(q[:], u_sb[:], rs[:], op=OP.mult)
              ent_junk = singles.tile([P, n_col], F32, tag="ent_junk")
              l1p = singles.tile([P, 1], F32, tag="l1p")
              _absorb(nc.vector, ab_v[:, 2 * GNT + rep:2 * GNT + rep + 1], [ln_s[:, 0:1]])
              # ent = ln_s - q ; l1p = sum over free
              nc.vector.scalar_tensor_tensor(
                  out=ent_junk[:], in0=ln_s[:], scalar=1.0, in1=q[:],
                  op0=OP.mult, op1=OP.subtract, accum_out=l1p[:])
              # partition reduce: ones^T @ l1p -> [1, 1]
              ps_l1 = psum.tile([1, 1], F32, tag="ps_l1")
              nc.tensor.matmul(ps_l1[0:1, 0:1], ones_f32[:], l1p[:],
                               start=True, stop=True)

              # ---- tail: block-mean entropies (core-local) -> L2 partial ----
              bm_sb = singles.tile([1, COLS], F32, tag="bm_sb")
              nc.scalar.mul(bm_sb[0:1, :], ps_cs[0:1, :], 1.0 / T)
              # ebm = exp(bm) written back over the psum colsum (saves SBUF)
              nc.scalar.activation(ps_cs[0:1, :], bm_sb[0:1, :], AF.Exp)
              _absorb(nc.vector, ab_t[0:1, 4 * rep:4 * rep + 2],
                      [bm_sb[0:1, 0:1], ps_cs[0:1, COLS - 1:COLS]])
              # tbm = bm * ebm, in place over bm_sb
              nc.vector.tensor_tensor(bm_sb[0:1, :], bm_sb[0:1, :],
                                      ps_cs[0:1, :], op=OP.mult)
              s_bm = singles.tile([1, M_LOC], F32, tag="s_bm")
              nc.vector.tensor_reduce(
                  out=s_bm[0:1, :], in_=ps_cs[0:1, :].rearrange("p (m k) -> p m k", k=K),
                  axis=mybir.AxisListType.X, op=OP.add)
              u_bm = singles.tile([1, M_LOC], F32, tag="u_bm")
              nc.vector.tensor_reduce(
                  out=u_bm[0:1, :], in_=bm_sb[0:1, :].rearrange("p (m k) -> p m k", k=K),
                  axis=mybir.AxisListType.X, op=OP.add)
              ln_sbm = singles.tile([1, M_LOC], F32, tag="ln_sbm")
              nc.scalar.activation(ln_sbm[0:1, :], s_bm[0:1, :], AF.Ln)
              r_sbm = singles.tile([1, M_LOC], F32, tag="r_sbm")
              nc.vector.reciprocal(r_sbm[0:1, :], s_bm[0:1, :])
              q_bm = singles.tile([1, M_LOC], F32, tag="q_bm")
              nc.vector.tensor_tensor(q_bm[0:1, :], u_bm[0:1, :], r_sbm[0:1, :],
                                      op=OP.mult)
              entbm_junk = singles.tile([1, M_LOC], F32, tag="entbm_junk")
              l2p = singles.tile([1, 1], F32, tag="l2p")
              _absorb(nc.vector, ab_t[0:1, 4 * rep + 2:4 * rep + 3], [ln_sbm[0:1, 0:1]])
              nc.vector.scalar_tensor_tensor(
                  out=entbm_junk[0:1, :], in0=ln_sbm[0:1, :], scalar=1.0,
                  in1=q_bm[0:1, :], op0=OP.mult, op1=OP.subtract,
                  accum_out=l2p[0:1, :])

              # ---- pack partials, AllReduce, final scalar ----
              cc_sb = singles.tile([1, 2], F32, tag="cc_sb")
              nc.scalar.copy(cc_sb[0:1, 0:1], ps_l1[0:1, 0:1])
              nc.scalar.copy(cc_sb[0:1, 1:2], l2p[0:1, 0:1])
              cc_res = singles.tile([1, 2], F32, tag="cc_res")
              if USE_COLLECTIVE:
                  cc_in = dram.tile([1, 2], F32, tag="cc_in")
                  cc_out = dram.tile([1, 2], F32, tag="cc_out")
                  nc.gpsimd.dma_start(cc_in[:], cc_sb[:])
                  nc.gpsimd.collective_compute(
                      "AllReduce", OP.add,
                      replica_groups=[list(range(N_CORES))],
                      ins=[cc_in.opt()], outs=[cc_out.opt()])
                  nc.sync.dma_start(cc_res[:], cc_out[:])
              else:
                  # per-core partials only; host sums the per-core outputs
                  nc.vector.tensor_copy(cc_res[:], cc_sb[:])

              t0 = singles.tile([1, 1], F32, tag="t0")
              nc.scalar.mul(t0[0:1, :], cc_res[0:1, 0:1], 1.0 / (T * M_TOT))
              t1 = singles.tile([1, 1], F32, tag="t1")
              nc.scalar.mul(t1[0:1, :], cc_res[0:1, 1:2], -LMBDA / M_TOT)
              out_sb = singles.tile([1, 1], F32, tag="out_sb")
              add_h = nc.vector.tensor_add(out_sb[0:1, :], t0[0:1, :], t1[0:1, :])
              hist[rep * NT + NT - 1]["tail_dve"] = add_h
              nc.sync.dma_start(out_dram.ap(), out_sb[:])

    nc.compile()
    return nc


_NC_CACHE = None


def _get_nc():
    global _NC_CACHE
    if _NC_CACHE is None:
        _NC_CACHE = build_nc()
    return _NC_CACHE


def _run(block_feats: np.ndarray, trace: bool = False):
    nc = _get_nc()
    x = np.asarray(block_feats, dtype=np.float32)
    assert x.shape == (T, N_CORES * COLS), x.shape
    in_maps = [
        {"x": np.ascontiguousarray(x[:, c * COLS:(c + 1) * COLS])}
        for c in range(N_CORES)
    ]
    res = run_bass_kernel_spmd(nc, in_maps, list(range(N_CORES)), trace=trace)
    val = np.float32(res.results[0]["out"][0, 0])
    return val, res


def kernel(block_feats: np.ndarray) -> np.ndarray:
    val, _ = _run(block_feats)
    return np.array(val, dtype=np.float32)


if __name__ == "__main__":
    rng = np.random.default_rng(0)
    xf = rng.standard_normal((T, N_CORES * COLS), dtype=np.float32)
    v = kernel(xf)
    print("kernel out:", v)



# revision 17
# speedup vs baseline: 1.0175x; 1.0175x over previous
"""ClusterLoss (mean-entropy + batch-entropy) Bass kernel for 8 trn2 cores.

Problem: block_feats [T=4096, M*K=64*256] f32.
  x = reshape(T, M, K)
  L1 = mean over (T, M) of entropy(softmax(x, axis=K))
  L2 = -sum_m entropy(softmax(mean_t x)) / M
  out = L1 + L2   (scalar)

Sharding: columns across 8 cores (each core: 8 blocks x all 4096 rows).
Single AllReduce of [1, 2] f32 combines per-core partials.

Per-core plan (rows on partitions, 8 super-tiles of [128, 4, 2048] bf16):
Each (row, block) needs s = sum_k e^x and u = sum_k x*e^x.  The s-accum
runs on DVE tensor_scalar (bf16 4x mode, 127ns/seg).  The u-pass is the
expensive part (no 4x op computes a two-tensor product-reduce), so it is
split per a-group across three methods to balance engines:
  'D': DVE tensor_tensor mult (2x) writes w = x*e in place over e, then
       TS-accum (4x) sums it.
  'P': the TT mult runs on the otherwise-idle Pool/GPSIMD engine
       (1x, 0.42 efficiency) into the e2 tile; DVE only does the accum.
  'F': centered finite difference: ACT computes e+ = exp((1+d/2)x) and
       e- = exp((1-d/2)x) (scale is free on ACT); TS-accum gives
       S+ and S-; then s = (S+ + S-)/2, u = (S+ - S-)/d with O(d^2)
       bias ~1e-5.  Costs a second ACT exp pass but no DVE mult.
Column sums for L2 run on PE (ones-matmul, dependency-gated so the
p-state model gives full clock) into a [4, 512] PSUM tile; the L2
entropy chain runs off the critical path right after the last matmul.
"""

import sys

sys.path.insert(0, "/opt/trn_rl_repo")

import numpy as np

import concourse.bass as bass
import concourse.bacc as bacc
import concourse.tile as tile
from concourse import mybir
from concourse.bass_utils import run_bass_kernel_spmd

F32 = mybir.dt.float32
BF16 = mybir.dt.bfloat16
AF = mybir.ActivationFunctionType
OP = mybir.AluOpType

# Problem constants
T = 4096            # rows (batch)
M_TOT = 64          # blocks
K = 256             # features per block
N_CORES = 8
COLS = (M_TOT * K) // N_CORES   # 2048 columns per core
M_LOC = COLS // K               # 8 blocks per core
P = 128                         # partitions
A = 4                           # row-groups packed per super-tile
ROWS_PER_TILE = P * A           # 512
NT = T // ROWS_PER_TILE         # 8 super-tiles

LMBDA = 1.0

# --- tuning knobs -----------------------------------------------------------
# Per-a-group u-pass method, indexed [it][a].  'P' = Pool TT, 'D' = DVE TT,
# 'F' = finite difference (second scaled exp on ACT).
AG_METHOD = [
    "PPPP",   # tile 0
    "PPPP",   # tile 1
    "PPPP",   # tile 2
    "DDDD",   # tile 3
    "DDDD",   # tile 4
    "PPFF",   # tile 5
    "FFFF",   # tile 6
    "FFFF",   # tile 7
]
DELTA = 2.0 ** -6
BUFS = 4             # rotation depth for x/e pools
BUFS2 = 3            # rotation depth for e2 pool (Pool-w / FD e-)
# Pool-tile deferred u-accum flush points: pool tile -> tile at whose DVE
# block the u-TS is emitted (late enough that the Pool TT chain has surely
# produced w; early enough not to pile into the tail).
POOL_FLUSH_AT = {0: 2, 1: 4, 2: 6, 5: 7}
USE_COLLECTIVE = True


def _absorb_deps(eng, dst_col, dep_insts):
    """Absorb cross-engine waits on `eng`'s queue before a wait-slot-limited
    instruction (e.g. SWDGE pseudo-DMA): one tiny input-free write per
    dependency, each carrying a single sem wait, advancing the engine's
    observed vector clock."""
    from concourse.tile_rust import add_dep_helper

    for j, di in enumerate(dep_insts):
        if hasattr(eng, "memset"):
            c = eng.memset(dst_col[:, j:j + 1], 0.0)
        else:
            c = eng.memzero(dst_col[:, j:j + 1])  # ScalarE
        add_dep_helper(c.ins, di.ins, reason="absorb wait for slot-limited op")


def _absorb(eng, dst_col, src_aps):
    """Absorb cross-engine waits: tiny copies that read the freshly produced
    tiles. Each copy carries one sem wait; once the engine has waited, its
    observed vector clock covers the tick, so following single-wait-slot
    instructions need no cross-engine waits."""
    for j, src in enumerate(src_aps):
        eng.tensor_copy(dst_col[:, j:j + 1], src)


def _runs(methods):
    """Contiguous same-method runs of an AG_METHOD string: [(m, a0, a1))."""
    out = []
    a = 0
    while a < len(methods):
        b = a
        while b < len(methods) and methods[b] == methods[a]:
            b += 1
        out.append((methods[a], a, b))
        a = b
    return out


def build_nc(reps: int = 1):
    assert reps == 1, "only reps=1 supported"
    nc = bacc.Bacc("TRN2", target_bir_lowering=False, debug=False,
                   num_devices=N_CORES)
    x_dram = nc.dram_tensor("x", [T, COLS], F32, kind="ExternalInput")
    out_dram = nc.dram_tensor("out", [1, 1], F32, kind="ExternalOutput")

    from contextlib import ExitStack

    sp = 1.0 + DELTA / 2.0
    sm = 1.0 - DELTA / 2.0

    with tile.TileContext(nc) as tc, ExitStack() as ctx:
        loads = ctx.enter_context(tc.tile_pool(name="loads", bufs=BUFS))
        es = ctx.enter_context(tc.tile_pool(name="es", bufs=BUFS))
        e2s = ctx.enter_context(tc.tile_pool(name="e2s", bufs=BUFS2))
        junks = ctx.enter_context(tc.tile_pool(name="junks", bufs=2))
        singles = ctx.enter_context(tc.tile_pool(name="singles", bufs=1))
        psum = ctx.enter_context(tc.tile_pool(name="psum", bufs=1, space="PSUM"))
        dram = ctx.enter_context(tc.tile_pool(name="dram", bufs=1, space="DRAM"))

        # persistent tiles
        ones_b = singles.tile([P, 1], BF16, tag="ones_b")      # matmul lhsT
        nc.vector.memset(ones_b, 1.0)
        ones_f32 = singles.tile([P, 1], F32, tag="ones_f32")
        nc.vector.memset(ones_f32, 1.0)
        s_sb = singles.tile([P, NT * A * M_LOC], F32, tag="s_sb")
        u_sb = singles.tile([P, NT * A * M_LOC], F32, tag="u_sb")
        fd_tmp = singles.tile([P, A * M_LOC], F32, tag="fd_tmp")
        # wait-absorber targets (disjoint columns per use)
        ab_v = singles.tile([P, 4 * NT + 8], F32, tag="ab_v")
        ab_dma = singles.tile([P, 6 * NT], F32, tag="ab_dma")
        ab_act = singles.tile([P, 2 * NT + 4], F32, tag="ab_act")
        ab_e2 = singles.tile([P, NT], F32, tag="ab_e2")
        ab_g = singles.tile([P, 2 * NT + 4], F32, tag="ab_g")
        ab_t = singles.tile([1, 4], F32, tag="ab_t")

        # activation-table warmup off the critical path (Exp + Ln)
        warm = singles.tile([1, 2], F32, tag="warm")
        nc.scalar.activation(warm[0:1, 0:1], ones_f32[0:1, 0:1], AF.Exp)
        nc.scalar.activation(warm[0:1, 1:2], ones_f32[0:1, 0:1], AF.Ln)

        # colsum accumulator in PSUM: [1, 2048] f32 (4 banks on partition 0;
        # matmul chunks of 512 stay within a bank)
        ps_cs = psum.tile([1, COLS], F32, tag="ps_cs")

        x_view = x_dram.ap().rearrange("(n a p) c -> n p a c", p=P, a=A)

        hist = {}           # it -> dict of instruction handles
        pend_pool = []      # deferred Pool-tile u-TS work: (it, a0, a1)
        e2_hist = []        # e2 pool allocation order (tile indices)

        def flush_pool_u(now_it):
            """Emit deferred u-accums for Pool tiles whose TT is surely done."""
            while pend_pool and POOL_FLUSH_AT.get(pend_pool[0][0], 99) <= now_it:
                pit, a0, a1 = pend_pool.pop(0)
                w_t = hist[pit]["e2"]
                # absorb the Pool-TT-done wait on the DVE queue
                _absorb(nc.vector, ab_v[:, 4 * pit + 2:4 * pit + 3],
                        [w_t[:, a0, 0:1]])
                for a in range(a0, a1):
                    for m in range(M_LOC):
                        idx = (pit * A + a) * M_LOC + m
                        sl = (slice(None), a, slice(m * K, (m + 1) * K))
                        h = nc.vector.tensor_scalar(
                            out=w_t[sl], in0=w_t[sl], scalar1=1.0,
                            scalar2=None, op0=OP.mult, op1=OP.add,
                            accum_out=u_sb[:, idx:idx + 1])
                hist[pit]["uTS"] = h

        for it in range(NT):
            methods = AG_METHOD[it]
            has_p = "P" in methods
            has_f = "F" in methods

            if it >= BUFS:
                # absorb WAR waits (readers of the recycled x/e slots) on the
                # Pool queue before the SWDGE DMA (1 wait slot).
                pv = hist[it - BUFS]
                deps = [pv[k] for k in ("act", "mm", "sTS", "uTS", "tt")
                        if k in pv]
                _absorb_deps(nc.gpsimd,
                             ab_dma[:, 6 * it:6 * it + len(deps)], deps)
            if (has_p or has_f) and len(e2_hist) >= BUFS2:
                # e2 slot recycles: absorb the old slot's readers on both
                # queues that will write it (Pool TT and/or ACT exp-)
                pv = hist[e2_hist[len(e2_hist) - BUFS2]]
                deps = [pv[k] for k in ("uTS",) if k in pv]
                if deps:
                    _absorb_deps(nc.gpsimd,
                                 ab_g[:, 2 * it:2 * it + len(deps)], deps)
                    _absorb_deps(nc.scalar,
                                 ab_e2[:, it:it + 1], deps)

            x_t = loads.tile([P, A, COLS], BF16, tag="x_t")
            if it == 0:
                # split the first load so ACT/DVE start ~3us earlier
                dma_h = nc.gpsimd.dma_start(out=x_t[:, 0:2], in_=x_view[0][:, 0:2])
                dma_h2 = nc.gpsimd.dma_start(out=x_t[:, 2:4], in_=x_view[0][:, 2:4])
            else:
                dma_h = nc.gpsimd.dma_start(out=x_t[:], in_=x_view[it])
                dma_h2 = dma_h
            hist[it] = {"dma": dma_h}

            e_t = es.tile([P, A, COLS], BF16, tag="e_t")
            e2_t = None
            if has_p or has_f:
                e2_t = e2s.tile([P, A, COLS], BF16, tag="e2_t")
                hist[it]["e2"] = e2_t
                e2_hist.append(it)

            # ---- ACT: exp passes --------------------------------------------
            # absorb the DMA-done wait on the ACT queue (1-wait-slot limit)
            _absorb_deps(nc.scalar, ab_act[:, 2 * it:2 * it + 1], [dma_h])
            if it == 0:
                _absorb_deps(nc.scalar, ab_act[:, 1:2], [dma_h2])
                act_h = nc.scalar.activation(e_t[:, 0:2], x_t[:, 0:2], AF.Exp)
                act_h = nc.scalar.activation(e_t[:, 2:4], x_t[:, 2:4], AF.Exp)
                hist[it]["act"] = act_h
            else:
                for meth, a0, a1 in _runs(methods):
                    if meth == "F":
                        h = nc.scalar.activation(e_t[:, a0:a1], x_t[:, a0:a1],
                                                 AF.Exp, scale=sp)
                        hist[it]["act"] = h
                        h2 = nc.scalar.activation(e2_t[:, a0:a1], x_t[:, a0:a1],
                                                  AF.Exp, scale=sm)
                        hist[it]["act2"] = h2
                    else:
                        h = nc.scalar.activation(e_t[:, a0:a1], x_t[:, a0:a1],
                                                 AF.Exp)
                        hist[it]["act"] = h

            # ---- Pool: TT mult for 'P' a-groups -----------------------------
            if has_p:
                for meth, a0, a1 in _runs(methods):
                    if meth != "P":
                        continue
                    # absorb exp-done (+ DMA for tile0's split) on Pool queue
                    dep = [hist[it]["act"]]
                    _absorb_deps(nc.gpsimd,
                                 ab_g[:, 2 * it + 1:2 * it + 2], dep)
                    tt_h = nc.gpsimd.tensor_tensor(
                        e2_t[:, a0:a1], x_t[:, a0:a1], e_t[:, a0:a1],
                        op=OP.mult)
                    hist[it]["tt"] = tt_h

            # ---- PE: colsum matmuls (gated on this tile's DMA) --------------
            for a in range(A):
                for c in range(COLS // 512):
                    mm_h = nc.tensor.matmul(
                        ps_cs[0:1, c * 512:(c + 1) * 512],
                        ones_b[:],
                        x_t[:, a, c * 512:(c + 1) * 512],
                        start=(it == 0 and a == 0),
                        stop=(it == NT - 1 and a == A - 1),
                    )
                    hist[it]["mm"] = mm_h

            # ---- DVE: s-accums (and D-tile TT + u-accums) -------------------
            flush_pool_u(it)
            # absorb exp-done + (x DMA for D tiles) + (exp- for F tiles) on
            # the DVE queue; the F absorb must read the exp- region, not a
            # region the Pool TT writes.
            ab_srcs = [e_t[:, 0, 0:1]]
            if "D" in methods:
                ab_srcs.append(x_t[:, 0, 0:1])
            if has_f:
                f_a0 = next(a0 for meth, a0, a1 in _runs(methods)
                            if meth == "F")
                ab_srcs.append(e2_t[:, f_a0, 0:1])
            _absorb(nc.vector, ab_v[:, 4 * it:4 * it + len(ab_srcs)], ab_srcs)

            junk = None
            if has_p:
                junk = junks.tile([P, COLS], BF16, tag="junk", name="junk")

            for meth, a0, a1 in _runs(methods):
                for a in range(a0, a1):
                    for m in range(M_LOC):
                        idx = (it * A + a) * M_LOC + m
                        sl = (slice(None), a, slice(m * K, (m + 1) * K))
                        jl = (slice(None), slice(m * K, (m + 1) * K))
                        # s = sum_k e (for F: S+)
                        out_ap = junk[jl] if meth == "P" else e_t[sl]
                        h = nc.vector.tensor_scalar(
                            out=out_ap, in0=e_t[sl], scalar1=1.0,
                            scalar2=None, op0=OP.mult, op1=OP.add,
                            accum_out=s_sb[:, idx:idx + 1])
                        hist[it]["sTS"] = h
                        if meth == "F":
                            # S- accum into u_sb (normalized below)
                            h = nc.vector.tensor_scalar(
                                out=e2_t[sl], in0=e2_t[sl], scalar1=1.0,
                                scalar2=None, op0=OP.mult, op1=OP.add,
                                accum_out=u_sb[:, idx:idx + 1])
                            hist[it]["uTS"] = h
                if meth == "D":
                    # w = x*e in place over e (2x TT), then accum each seg
                    tt_h = nc.vector.tensor_tensor(
                        e_t[:, a0:a1], x_t[:, a0:a1], e_t[:, a0:a1],
                        op=OP.mult)
                    hist[it]["tt"] = tt_h
                    for a in range(a0, a1):
                        for m in range(M_LOC):
                            idx = (it * A + a) * M_LOC + m
                            sl = (slice(None), a, slice(m * K, (m + 1) * K))
                            h = nc.vector.tensor_scalar(
                                out=e_t[sl], in0=e_t[sl], scalar1=1.0,
                                scalar2=None, op0=OP.mult, op1=OP.add,
                                accum_out=u_sb[:, idx:idx + 1])
                            hist[it]["uTS"] = h
                if meth == "P":
                    pend_pool.append((it, a0, a1))

            # FD normalization for this tile's F columns:
            #   s <- (S+ + S-)/2 ; u <- (S+ - S-)/DELTA
            for meth, a0, a1 in _runs(methods):
                if meth != "F":
                    continue
                c0 = (it * A + a0) * M_LOC
                c1 = (it * A + a1) * M_LOC
                n = c1 - c0
                nc.vector.tensor_tensor(
                    fd_tmp[:, 0:n], s_sb[:, c0:c1], u_sb[:, c0:c1],
                    op=OP.subtract)
                nc.vector.tensor_tensor(
                    s_sb[:, c0:c1], s_sb[:, c0:c1], u_sb[:, c0:c1],
                    op=OP.add)
                nc.vector.tensor_scalar(
                    out=s_sb[:, c0:c1], in0=s_sb[:, c0:c1], scalar1=0.5,
                    scalar2=None, op0=OP.mult, op1=OP.add)
                nc.vector.tensor_scalar(
                    out=u_sb[:, c0:c1], in0=fd_tmp[:, 0:n],
                    scalar1=1.0 / DELTA,
                    scalar2=None, op0=OP.mult, op1=OP.add)

        # flush any remaining deferred Pool u-accums
        while pend_pool:
            pit, a0, a1 = pend_pool.pop(0)
            w_t = hist[pit]["e2"]
            _absorb(nc.vector, ab_v[:, 4 * pit + 2:4 * pit + 3],
                    [w_t[:, a0, 0:1]])
            for a in range(a0, a1):
                for m in range(M_LOC):
                    idx = (pit * A + a) * M_LOC + m
                    sl = (slice(None), a, slice(m * K, (m + 1) * K))
                    nc.vector.tensor_scalar(
                        out=w_t[sl], in0=w_t[sl], scalar1=1.0,
                        scalar2=None, op0=OP.mult, op1=OP.add,
                        accum_out=u_sb[:, idx:idx + 1])

        # ---- L2 chain: block-mean entropies (off critical path) -------------
        # Scatter the [1, 2048] colsum row to [8, 256] (one block per
        # partition) with an SBUF->SBUF DMA on the idle SP queue, then the
        # whole chain is a few short 8-partition ops.
        n_col = NT * A * M_LOC  # 256
        cs_row = singles.tile([1, COLS], F32, tag="cs_row")
        nc.gpsimd.tensor_copy(cs_row[0:1, :], ps_cs[0:1, :])
        cs_sb = singles.tile([8, K], F32, tag="cs_sb")
        nc.sync.dma_start(
            out=cs_sb[0:8, :],
            in_=cs_row[0:1, :].rearrange("p (m k) -> (p m) k", k=K))
        ebm = singles.tile([8, K], BF16, tag="ebm")
        nc.scalar.activation(ebm[0:8, :], cs_sb[0:8, :], AF.Exp,
                             scale=1.0 / T)
        tbm = singles.tile([8, K], BF16, tag="tbm")
        # tbm = cs * ebm  (the 1/T of bm folded in at the entropy STT below)
        nc.vector.tensor_tensor(tbm[0:8, :], cs_sb[0:8, :], ebm[0:8, :],
                                op=OP.mult)
        s_bm = singles.tile([8, 1], F32, tag="s_bm")
        u_bm = singles.tile([8, 1], F32, tag="u_bm")
        nc.vector.tensor_scalar(
            out=ebm[0:8, :], in0=ebm[0:8, :],
            scalar1=1.0, scalar2=None, op0=OP.mult, op1=OP.add,
            accum_out=s_bm[0:8, :])
        nc.vector.tensor_scalar(
            out=tbm[0:8, :], in0=tbm[0:8, :],
            scalar1=1.0, scalar2=None, op0=OP.mult, op1=OP.add,
            accum_out=u_bm[0:8, :])
        ln_sbm = singles.tile([8, 1], F32, tag="ln_sbm")
        nc.scalar.activation(ln_sbm[0:8, :], s_bm[0:8, :], AF.Ln)
        r_sbm = singles.tile([8, 1], F32, tag="r_sbm")
        nc.vector.reciprocal(r_sbm[0:8, :], s_bm[0:8, :])
        q_bm = singles.tile([8, 1], F32, tag="q_bm")
        nc.vector.tensor_tensor(q_bm[0:8, :], u_bm[0:8, :], r_sbm[0:8, :],
                                op=OP.mult)
        # q_bm holds (sum cs*ebm)/s; the true u/s = q_bm/T
        entbm_junk = singles.tile([8, 1], F32, tag="entbm_junk")
        l2p_col = singles.tile([8, 1], F32, tag="l2p_col")
        _absorb(nc.vector, ab_t[0:1, 0:1], [ln_sbm[0:1, 0:1]])
        nc.vector.scalar_tensor_tensor(
            out=entbm_junk[0:8, :], in0=q_bm[0:8, :], scalar=1.0 / T,
            in1=ln_sbm[0:8, :], op0=OP.mult, op1=OP.subtract,
            accum_out=l2p_col[0:8, :])
        # l2p_col = (q/T - ln s) per partition  == -entropy per block
        ps_l2 = psum.tile([1, 1], F32, tag="ps_l2")
        nc.tensor.matmul(ps_l2[0:1, 0:1], ones_f32[0:8], l2p_col[0:8, :],
                         start=True, stop=True)

        # ---- L1 tail: ent = ln s - u/s over all 256 cols --------------------
        ln_s = singles.tile([P, n_col], F32, tag="ln_s")
        nc.scalar.activation(ln_s[:], s_sb[:], AF.Ln)
        rs = singles.tile([P, n_col], F32, tag="rs")
        nc.vector.reciprocal_approx_fast(rs[:], s_sb[:])
        q = singles.tile([P, n_col], F32, tag="q")
        nc.vector.tensor_tensor(q[:], u_sb[:], rs[:], op=OP.mult)
        ent_junk = singles.tile([P, n_col], F32, tag="ent_junk")
        l1p = singles.tile([P, 1], F32, tag="l1p")
        _absorb(nc.vector, ab_v[:, 4 * NT + 1:4 * NT + 2], [ln_s[:, 0:1]])
        nc.vector.scalar_tensor_tensor(
            out=ent_junk[:], in0=ln_s[:], scalar=1.0, in1=q[:],
            op0=OP.mult, op1=OP.subtract, accum_out=l1p[:])
        ps_l1 = psum.tile([1, 1], F32, tag="ps_l1")
        nc.tensor.matmul(ps_l1[0:1, 0:1], ones_f32[:], l1p[:],
                         start=True, stop=True)

        # ---- pack partials, AllReduce, final scalar -------------------------
        cc_sb = singles.tile([1, 2], F32, tag="cc_sb")
        nc.scalar.copy(cc_sb[0:1, 0:1], ps_l1[0:1, 0:1])
        nc.scalar.copy(cc_sb[0:1, 1:2], ps_l2[0:1, 0:1])
        cc_res = singles.tile([1, 2], F32, tag="cc_res")
        if USE_COLLECTIVE:
            cc_in = dram.tile([1, 2], F32, tag="cc_in")
            cc_out = dram.tile([1, 2], F32, tag="cc_out")
            nc.gpsimd.dma_start(cc_in[:], cc_sb[:])
            nc.gpsimd.collective_compute(
                "AllReduce", OP.add,
                replica_groups=[list(range(N_CORES))],
                ins=[cc_in.opt()], outs=[cc_out.opt()])
            nc.sync.dma_start(cc_res[:], cc_out[:])
        else:
            nc.vector.tensor_copy(cc_res[:], cc_sb[:])

        t0 = singles.tile([1, 1], F32, tag="t0")
        nc.scalar.mul(t0[0:1, :], cc_res[0:1, 0:1], 1.0 / (T * M_TOT))
        t1 = singles.tile([1, 1], F32, tag="t1")
        # l2p already holds -entropy partials, so L2 = +sum/M
        nc.scalar.mul(t1[0:1, :], cc_res[0:1, 1:2], LMBDA / M_TOT)
        out_sb = singles.tile([1, 1], F32, tag="out_sb")
        nc.vector.tensor_add(out_sb[0:1, :], t0[0:1, :], t1[0:1, :])
        nc.sync.dma_start(out_dram.ap(), out_sb[:])

    nc.compile()
    return nc


_NC_CACHE = None


def _get_nc():
    global _NC_CACHE
    if _NC_CACHE is None:
        _NC_CACHE = build_nc()
    return _NC_CACHE


def _run(block_feats: np.ndarray, trace: bool = False):
    nc = _get_nc()
    x = np.asarray(block_feats, dtype=np.float32)
    assert x.shape == (T, N_CORES * COLS), x.shape
    in_maps = [
        {"x": np.ascontiguousarray(x[:, c * COLS:(c + 1) * COLS])}
        for c in range(N_CORES)
    ]
    res = run_bass_kernel_spmd(nc, in_maps, list(range(N_CORES)), trace=trace)
    val = np.float32(res.results[0]["out"][0, 0])
    return val, res


def kernel(block_feats: np.ndarray) -> np.ndarray:
    val, _ = _run(block_feats)
    return np.array(val, dtype=np.float32)


if __name__ == "__main__":
    rng = np.random.default_rng(0)
    xf = rng.standard_normal((T, N_CORES * COLS), dtype=np.float32)
    v = kernel(xf)
    print("kernel out:", v)


# revision 23
# speedup vs baseline: 1.2744x; 1.2525x over previous
"""ClusterLoss (mean-entropy + batch-entropy) Bass kernel for 8 trn2 cores.

Problem: block_feats [T=4096, M*K=64*256] f32.
  x = reshape(T, M, K)
  L1 = mean over (T, M) of entropy(softmax(x, axis=K))
  L2 = -sum_m entropy(softmax(mean_t x)) / M
  out = L1 + L2   (scalar)

Sharding: columns across 8 cores (each core: 8 blocks x all 4096 rows).
Single AllReduce of [1, 2] f32 combines per-core partials.

Per-core plan (rows on partitions, 8 super-tiles of [128, 4, 2048] bf16):
Each (row, block) needs s = sum_k e^x and u = sum_k x*e^x.  The s-accum
runs on DVE tensor_scalar (bf16 4x mode, 127ns/seg).  The u-pass is the
expensive part (no 4x op computes a two-tensor product-reduce), so it is
split per a-group across three methods to balance engines:
  'D': DVE tensor_tensor mult (2x) writes w = x*e in place over e, then
       TS-accum (4x) sums it.
  'P': the TT mult runs on the otherwise-idle Pool/GPSIMD engine
       (1x, 0.42 efficiency) into the e2 tile; DVE only does the accum.
  'F': centered finite difference: ACT computes e+ = exp((1+d/2)x) and
       e- = exp((1-d/2)x) (scale is free on ACT); TS-accum gives
       S+ and S-; then s = (S+ + S-)/2, u = (S+ - S-)/d with O(d^2)
       bias ~1e-5.  Costs a second ACT exp pass but no DVE mult.
Column sums for L2 run on PE (ones-matmul, dependency-gated so the
p-state model gives full clock) into a [4, 512] PSUM tile; the L2
entropy chain runs off the critical path right after the last matmul.
"""

import sys

sys.path.insert(0, "/opt/trn_rl_repo")

import numpy as np

import concourse.bass as bass
import concourse.bacc as bacc
import concourse.tile as tile
from concourse import mybir
from concourse.bass_utils import run_bass_kernel_spmd

F32 = mybir.dt.float32
BF16 = mybir.dt.bfloat16
AF = mybir.ActivationFunctionType
OP = mybir.AluOpType

# Problem constants
T = 4096            # rows (batch)
M_TOT = 64          # blocks
K = 256             # features per block
N_CORES = 8
COLS = (M_TOT * K) // N_CORES   # 2048 columns per core
M_LOC = COLS // K               # 8 blocks per core
P = 128                         # partitions
A = 4                           # row-groups packed per super-tile
ROWS_PER_TILE = P * A           # 512
NT = T // ROWS_PER_TILE         # 8 super-tiles

LMBDA = 1.0

# --- tuning knobs -----------------------------------------------------------
# Per-a-group u-pass method, indexed [it][a].  'P' = Pool TT, 'D' = DVE TT,
# 'F' = finite difference (second scaled exp on ACT).
AG_METHOD = [
    "PPDD",   # tile 0
    "PPDD",   # tile 1
    "PPDD",   # tile 2
    "PPDD",   # tile 3
    "PPFF",   # tile 4
    "PPFF",   # tile 5
    "PPFF",   # tile 6
    "FFFF",   # tile 7
]
DELTA = 2.0 ** -6
BUFS = 4             # rotation depth for x/e pools
BUFS2 = 3            # rotation depth for e2 pool (Pool-w / FD e-)
# Pool-tile deferred u-accum flush points: pool tile -> tile at whose DVE
# block the u-TS is emitted (late enough that the Pool TT chain has surely
# produced w; early enough not to pile into the tail).
POOL_FLUSH_AT = {0: 2, 1: 3, 2: 4, 3: 5, 4: 6, 5: 7, 6: 7}
USE_COLLECTIVE = True


def _absorb_deps(eng, dst_col, dep_insts):
    """Absorb cross-engine waits on `eng`'s queue before a wait-slot-limited
    instruction (e.g. SWDGE pseudo-DMA): one tiny input-free write per
    dependency, each carrying a single sem wait, advancing the engine's
    observed vector clock."""
    from concourse.tile_rust import add_dep_helper

    for j, di in enumerate(dep_insts):
        if hasattr(eng, "memset"):
            c = eng.memset(dst_col[:, j:j + 1], 0.0)
        else:
            c = eng.memzero(dst_col[:, j:j + 1])  # ScalarE
        add_dep_helper(c.ins, di.ins, reason="absorb wait for slot-limited op")


def _absorb(eng, dst_col, src_aps):
    """Absorb cross-engine waits: tiny copies that read the freshly produced
    tiles. Each copy carries one sem wait; once the engine has waited, its
    observed vector clock covers the tick, so following single-wait-slot
    instructions need no cross-engine waits."""
    for j, src in enumerate(src_aps):
        eng.tensor_copy(dst_col[:, j:j + 1], src)


def _runs(methods):
    """Contiguous same-method runs of an AG_METHOD string: [(m, a0, a1))."""
    out = []
    a = 0
    while a < len(methods):
        b = a
        while b < len(methods) and methods[b] == methods[a]:
            b += 1
        out.append((methods[a], a, b))
        a = b
    return out


def build_nc(reps: int = 1):
    assert reps == 1, "only reps=1 supported"
    nc = bacc.Bacc("TRN2", target_bir_lowering=False, debug=False,
                   num_devices=N_CORES)
    x_dram = nc.dram_tensor("x", [T, COLS], F32, kind="ExternalInput")
    out_dram = nc.dram_tensor("out", [1, 1], F32, kind="ExternalOutput")

    from contextlib import ExitStack

    sp = 1.0 + DELTA / 2.0
    sm = 1.0 - DELTA / 2.0

    with tile.TileContext(nc) as tc, ExitStack() as ctx:
        loads = ctx.enter_context(tc.tile_pool(name="loads", bufs=BUFS))
        es = ctx.enter_context(tc.tile_pool(name="es", bufs=BUFS))
        e2s = ctx.enter_context(tc.tile_pool(name="e2s", bufs=BUFS2))
        junks = ctx.enter_context(tc.tile_pool(name="junks", bufs=2))
        singles = ctx.enter_context(tc.tile_pool(name="singles", bufs=1))
        psum = ctx.enter_context(tc.tile_pool(name="psum", bufs=1, space="PSUM"))
        dram = ctx.enter_context(tc.tile_pool(name="dram", bufs=1, space="DRAM"))

        # persistent tiles
        ones_b = singles.tile([P, 1], BF16, tag="ones_b")      # matmul lhsT
        nc.vector.memset(ones_b, 1.0)
        ones_f32 = singles.tile([P, 1], F32, tag="ones_f32")
        nc.vector.memset(ones_f32, 1.0)
        s_sb = singles.tile([P, NT * A * M_LOC], F32, tag="s_sb")
        u_sb = singles.tile([P, NT * A * M_LOC], F32, tag="u_sb")
        fd_tmp = singles.tile([P, A * M_LOC], F32, tag="fd_tmp")
        # wait-absorber targets (disjoint columns per use)
        ab_v = singles.tile([P, 4 * NT + 8], F32, tag="ab_v")
        ab_dma = singles.tile([P, 6 * NT], F32, tag="ab_dma")
        ab_act = singles.tile([P, 2 * NT + 4], F32, tag="ab_act")
        ab_e2 = singles.tile([P, NT], F32, tag="ab_e2")
        ab_g = singles.tile([P, 2 * NT + 4], F32, tag="ab_g")
        ab_act2 = singles.tile([P, 2 * NT], F32, tag="ab_act2")
        ab_t = singles.tile([1, 4], F32, tag="ab_t")

        # activation-table warmup off the critical path (Exp + Ln)
        warm = singles.tile([1, 2], F32, tag="warm")
        nc.scalar.activation(warm[0:1, 0:1], ones_f32[0:1, 0:1], AF.Exp)
        nc.scalar.activation(warm[0:1, 1:2], ones_f32[0:1, 0:1], AF.Ln)

        # colsum accumulator in PSUM: [1, 2048] f32 (4 banks on partition 0;
        # matmul chunks of 512 stay within a bank)
        ps_cs = psum.tile([1, COLS], F32, tag="ps_cs")

        x_view = x_dram.ap().rearrange("(n a p) c -> n p a c", p=P, a=A)

        hist = {t: {} for t in range(NT)}
        pend_pool = []      # deferred Pool-tile u-TS work: (it, a0, a1)

        def issue_dma(t):
            """SWDGE bf16 cast-load for tile t (split for tile 0)."""
            x_t = loads.tile([P, A, COLS], BF16, tag="x_t", name=f"x{t}")
            hist[t]["x"] = x_t
            if t == 0:
                h = nc.gpsimd.dma_start(out=x_t[:, 0:2], in_=x_view[0][:, 0:2])
                hist[t]["dma"] = h
                h2 = nc.gpsimd.dma_start(out=x_t[:, 2:4], in_=x_view[0][:, 2:4])
                hist[t]["dma2"] = h2
            else:
                h = nc.gpsimd.dma_start(out=x_t[:], in_=x_view[t])
                hist[t]["dma"] = h

        def issue_mms(t):
            """Colsum ones-matmuls for tile t (gated on its DMA)."""
            x_t = hist[t]["x"]
            for a in range(A):
                for c in range(COLS // 512):
                    mm_h = nc.tensor.matmul(
                        ps_cs[0:1, c * 512:(c + 1) * 512],
                        ones_b[:],
                        x_t[:, a, c * 512:(c + 1) * 512],
                        start=(t == 0 and a == 0),
                        stop=(t == NT - 1 and a == A - 1),
                    )
                    hist[t]["mm"] = mm_h

        def flush_pool_u(now_it):
            """Emit deferred u-accums for Pool tiles whose TT is surely done."""
            while pend_pool and POOL_FLUSH_AT.get(pend_pool[0][0], 99) <= now_it:
                pit, a0, a1 = pend_pool.pop(0)
                w_t = hist[pit]["e2"]
                # absorb the Pool-TT-done wait on the DVE queue
                _absorb(nc.vector, ab_v[:, 4 * pit + 2:4 * pit + 3],
                        [w_t[:, a0, 0:1]])
                h = None
                for a in range(a0, a1):
                    for m in range(M_LOC):
                        idx = (pit * A + a) * M_LOC + m
                        sl = (slice(None), a, slice(m * K, (m + 1) * K))
                        h = nc.vector.tensor_scalar(
                            out=w_t[sl], in0=w_t[sl], scalar1=1.0,
                            scalar2=None, op0=OP.mult, op1=OP.add,
                            accum_out=u_sb[:, idx:idx + 1])
                hist[pit]["uTS"] = h

        # L2 chain pieces (emitted mid-loop to keep them off the critical
        # path): part A evacuates+scatters the colsum, part B computes the
        # per-block entropies.
        l2 = {}

        def l2_part_a():
            cs_row = singles.tile([1, COLS], F32, tag="cs_row")
            nc.gpsimd.tensor_copy(cs_row[0:1, :], ps_cs[0:1, :])
            cs_sb = singles.tile([8, K], F32, tag="cs_sb")
            nc.sync.dma_start(
                out=cs_sb[0:8, :],
                in_=cs_row[0:1, :].rearrange("p (m k) -> (p m) k", k=K))
            l2["cs_sb"] = cs_sb

        def l2_part_b():
            cs_sb = l2["cs_sb"]
            ebm = singles.tile([8, K], BF16, tag="ebm")
            nc.scalar.activation(ebm[0:8, :], cs_sb[0:8, :], AF.Exp,
                                 scale=1.0 / T)
            tbm = singles.tile([8, K], BF16, tag="tbm")
            # tbm = cs * ebm  (1/T folded into the entropy STT below)
            _absorb(nc.vector, ab_t[0:1, 1:2], [ebm[0:1, 0:1]])
            nc.vector.tensor_tensor(tbm[0:8, :], cs_sb[0:8, :], ebm[0:8, :],
                                    op=OP.mult)
            s_bm = singles.tile([8, 1], F32, tag="s_bm")
            u_bm = singles.tile([8, 1], F32, tag="u_bm")
            nc.vector.tensor_scalar(
                out=ebm[0:8, :], in0=ebm[0:8, :],
                scalar1=1.0, scalar2=None, op0=OP.mult, op1=OP.add,
                accum_out=s_bm[0:8, :])
            nc.vector.tensor_scalar(
                out=tbm[0:8, :], in0=tbm[0:8, :],
                scalar1=1.0, scalar2=None, op0=OP.mult, op1=OP.add,
                accum_out=u_bm[0:8, :])
            ln_sbm = singles.tile([8, 1], F32, tag="ln_sbm")
            nc.scalar.activation(ln_sbm[0:8, :], s_bm[0:8, :], AF.Ln)
            r_sbm = singles.tile([8, 1], F32, tag="r_sbm")
            nc.vector.reciprocal(r_sbm[0:8, :], s_bm[0:8, :])
            q_bm = singles.tile([8, 1], F32, tag="q_bm")
            nc.vector.tensor_tensor(q_bm[0:8, :], u_bm[0:8, :],
                                    r_sbm[0:8, :], op=OP.mult)
            entbm_junk = singles.tile([8, 1], F32, tag="entbm_junk")
            l2p_col = singles.tile([8, 1], F32, tag="l2p_col")
            _absorb(nc.vector, ab_t[0:1, 0:1], [ln_sbm[0:1, 0:1]])
            nc.vector.scalar_tensor_tensor(
                out=entbm_junk[0:8, :], in0=q_bm[0:8, :], scalar=1.0 / T,
                in1=ln_sbm[0:8, :], op0=OP.mult, op1=OP.subtract,
                accum_out=l2p_col[0:8, :])
            # l2p_col = (q/T - ln s) per partition  == -entropy per block
            ps_l2 = psum.tile([1, 1], F32, tag="ps_l2")
            nc.tensor.matmul(ps_l2[0:1, 0:1], ones_f32[0:8],
                             l2p_col[0:8, :], start=True, stop=True)
            l2["ps_l2"] = ps_l2

        # prime the pipeline: first BUFS loads before any Pool TT
        for t in range(BUFS):
            issue_dma(t)

        for it in range(NT):
            methods = AG_METHOD[it]
            has_p = "P" in methods
            has_f = "F" in methods
            x_t = hist[it]["x"]

            # ---- ACT: exp passes -------------------------------------------
            if it >= 1:
                # e-slot WAR: absorb old e readers on the scalar queue
                pv = hist.get(it - BUFS, {})
                deps = [pv[k] for k in ("sTS", "uTS") if k in pv]
                if deps:
                    _absorb_deps(nc.scalar,
                                 ab_act[:, 2 * it:2 * it + len(deps)], deps)
            _absorb_deps(nc.scalar, ab_act2[:, 2 * it:2 * it + 1],
                         [hist[it]["dma"]])
            if "dma2" in hist[it]:
                _absorb_deps(nc.scalar, ab_act2[:, 2 * it + 1:2 * it + 2],
                             [hist[it]["dma2"]])
            e_t = es.tile([P, A, COLS], BF16, tag="e_t", name=f"e{it}")
            hist[it]["e"] = e_t
            # e2 slot WAR: absorb old slot readers before its writers
            e2_t = None
            if has_p or has_f:
                pv = hist.get(it - BUFS2, {})
                if "e2" in pv and "uTS" in pv:
                    _absorb_deps(nc.scalar, ab_e2[:, it:it + 1], [pv["uTS"]])
                    _absorb_deps(nc.gpsimd, ab_g[:, 2 * it:2 * it + 1],
                                 [pv["uTS"]])
                e2_t = e2s.tile([P, A, COLS], BF16, tag="e2_t",
                                name=f"e2{it}")
                hist[it]["e2"] = e2_t

            # plain exp covers the contiguous P/D prefix in one op; F spans
            # get the scaled pair.  Tile 0 splits its plain exp for pipeline
            # fill; the last tile splits its F pairs per a-group so the final
            # accums chase each exp.
            n_plain = sum(1 for c in methods if c in "PD")
            if it == 0:
                h = nc.scalar.activation(e_t[:, 0:2], x_t[:, 0:2], AF.Exp)
                hist[it]["act_p"] = h
                h = nc.scalar.activation(e_t[:, 2:4], x_t[:, 2:4], AF.Exp)
                hist[it]["act"] = h
            elif it == NT - 1 and methods == "FFFF":
                for a in range(A):
                    h = nc.scalar.activation(e_t[:, a:a + 1], x_t[:, a:a + 1],
                                             AF.Exp, scale=sp)
                    hist[it]["act"] = h
                    h2 = nc.scalar.activation(e2_t[:, a:a + 1],
                                              x_t[:, a:a + 1], AF.Exp,
                                              scale=sm)
                    hist[it]["act2"] = h2
            else:
                if n_plain:
                    h = nc.scalar.activation(e_t[:, 0:n_plain],
                                             x_t[:, 0:n_plain], AF.Exp)
                    hist[it]["act_p"] = h
                    hist[it]["act"] = h
                if n_plain < A:
                    h = nc.scalar.activation(e_t[:, n_plain:A],
                                             x_t[:, n_plain:A], AF.Exp,
                                             scale=sp)
                    hist[it]["act"] = h
                    h2 = nc.scalar.activation(e2_t[:, n_plain:A],
                                              x_t[:, n_plain:A], AF.Exp,
                                              scale=sm)
                    hist[it]["act2"] = h2

            # ---- Pool: TT mult for 'P' a-groups ----------------------------
            if has_p:
                for meth, a0, a1 in _runs(methods):
                    if meth != "P":
                        continue
                    _absorb_deps(nc.gpsimd, ab_g[:, 2 * it + 1:2 * it + 2],
                                 [hist[it]["act_p"]])
                    tt_h = nc.gpsimd.tensor_tensor(
                        e2_t[:, a0:a1], x_t[:, a0:a1], e_t[:, a0:a1],
                        op=OP.mult)
                    hist[it]["ptt"] = tt_h
                    pend_pool.append((it, a0, a1))

            # ---- Pool: next SWDGE load (after this tile's TT) --------------
            nt_ = it + BUFS - 1
            if 1 <= it and nt_ < NT:
                pv = hist[it - 1]
                deps = [pv[k] for k in ("act", "mm", "tt", "sTS", "uTS")
                        if k in pv]
                _absorb_deps(nc.gpsimd,
                             ab_dma[:, 6 * it:6 * it + len(deps)], deps)
                issue_dma(nt_)

            # ---- PE: colsum matmuls ----------------------------------------
            if it <= NT - 3:
                issue_mms(it)
            if it == NT - 3:
                issue_mms(NT - 2)
                issue_mms(NT - 1)

            # ---- DVE: accums (and D-tile TT + u-accums) --------------------
            flush_pool_u(it)
            ab_srcs = [e_t[:, 0, 0:1]]
            if "D" in methods:
                ab_srcs.append(x_t[:, 0, 0:1])
            if has_f:
                f_a0 = next(a0 for meth, a0, a1 in _runs(methods)
                            if meth == "F")
                ab_srcs.append(e2_t[:, f_a0, 0:1])
            _absorb(nc.vector, ab_v[:, 4 * it:4 * it + len(ab_srcs)], ab_srcs)

            junk = None
            if has_p:
                junk = junks.tile([P, COLS], BF16, tag="junk", name=f"j{it}")

            for meth, a0, a1 in _runs(methods):
                h = None
                for a in range(a0, a1):
                    for m in range(M_LOC):
                        idx = (it * A + a) * M_LOC + m
                        sl = (slice(None), a, slice(m * K, (m + 1) * K))
                        jl = (slice(None), slice(m * K, (m + 1) * K))
                        # s = sum_k e (for F: S+)
                        out_ap = junk[jl] if meth == "P" else e_t[sl]
                        h = nc.vector.tensor_scalar(
                            out=out_ap, in0=e_t[sl], scalar1=1.0,
                            scalar2=None, op0=OP.mult, op1=OP.add,
                            accum_out=s_sb[:, idx:idx + 1])
                        hist[it]["sTS"] = h
                        if meth == "F":
                            # S- accum into u_sb (normalized below)
                            h = nc.vector.tensor_scalar(
                                out=e2_t[sl], in0=e2_t[sl], scalar1=1.0,
                                scalar2=None, op0=OP.mult, op1=OP.add,
                                accum_out=u_sb[:, idx:idx + 1])
                            hist[it]["uTS"] = h
                if meth == "D":
                    # w = x*e in place over e (2x TT), then accum each seg
                    tt_h = nc.vector.tensor_tensor(
                        e_t[:, a0:a1], x_t[:, a0:a1], e_t[:, a0:a1],
                        op=OP.mult)
                    hist[it]["tt"] = tt_h
                    for a in range(a0, a1):
                        for m in range(M_LOC):
                            idx = (it * A + a) * M_LOC + m
                            sl = (slice(None), a, slice(m * K, (m + 1) * K))
                            h = nc.vector.tensor_scalar(
                                out=e_t[sl], in0=e_t[sl], scalar1=1.0,
                                scalar2=None, op0=OP.mult, op1=OP.add,
                                accum_out=u_sb[:, idx:idx + 1])
                            hist[it]["uTS"] = h

            # FD normalization for this tile's F columns:
            #   s <- (S+ + S-)/2 ; u <- (S+ - S-)/DELTA
            for meth, a0, a1 in _runs(methods):
                if meth != "F":
                    continue
                c0 = (it * A + a0) * M_LOC
                c1 = (it * A + a1) * M_LOC
                n = c1 - c0
                nc.vector.tensor_tensor(
                    fd_tmp[:, 0:n], s_sb[:, c0:c1], u_sb[:, c0:c1],
                    op=OP.subtract)
                nc.vector.tensor_tensor(
                    s_sb[:, c0:c1], s_sb[:, c0:c1], u_sb[:, c0:c1],
                    op=OP.add)
                nc.vector.tensor_scalar(
                    out=s_sb[:, c0:c1], in0=s_sb[:, c0:c1], scalar1=0.5,
                    scalar2=None, op0=OP.mult, op1=OP.add)
                nc.vector.tensor_scalar(
                    out=u_sb[:, c0:c1], in0=fd_tmp[:, 0:n],
                    scalar1=1.0 / DELTA,
                    scalar2=None, op0=OP.mult, op1=OP.add)

            # ---- L2 chain, interleaved off the critical path ---------------
            if it == NT - 3:
                l2_part_a()
            if it == NT - 2:
                l2_part_b()

        # flush any remaining deferred Pool u-accums
        flush_pool_u(99)
        ps_l2 = l2["ps_l2"]

        # ---- L1 tail: ent = ln s - u/s over all 256 cols --------------------
        n_col = NT * A * M_LOC  # 256
        ln_s = singles.tile([P, n_col], F32, tag="ln_s")
        nc.scalar.activation(ln_s[:], s_sb[:], AF.Ln)
        rs = singles.tile([P, n_col], F32, tag="rs")
        nc.vector.reciprocal_approx_fast(rs[:], s_sb[:])
        q = singles.tile([P, n_col], F32, tag="q")
        nc.vector.tensor_tensor(q[:], u_sb[:], rs[:], op=OP.mult)
        ent_junk = singles.tile([P, n_col], F32, tag="ent_junk")
        l1p = singles.tile([P, 1], F32, tag="l1p")
        _absorb(nc.vector, ab_v[:, 4 * NT + 1:4 * NT + 2], [ln_s[:, 0:1]])
        nc.vector.scalar_tensor_tensor(
            out=ent_junk[:], in0=ln_s[:], scalar=1.0, in1=q[:],
            op0=OP.mult, op1=OP.subtract, accum_out=l1p[:])
        ps_l1 = psum.tile([1, 1], F32, tag="ps_l1")
        nc.tensor.matmul(ps_l1[0:1, 0:1], ones_f32[:], l1p[:],
                         start=True, stop=True)

        # ---- pack partials, AllReduce, final scalar -------------------------
        cc_sb = singles.tile([1, 2], F32, tag="cc_sb")
        nc.scalar.copy(cc_sb[0:1, 0:1], ps_l1[0:1, 0:1])
        nc.scalar.copy(cc_sb[0:1, 1:2], ps_l2[0:1, 0:1])
        cc_res = singles.tile([1, 2], F32, tag="cc_res")
        if USE_COLLECTIVE:
            cc_in = dram.tile([1, 2], F32, tag="cc_in")
            cc_out = dram.tile([1, 2], F32, tag="cc_out")
            nc.gpsimd.dma_start(cc_in[:], cc_sb[:])
            nc.gpsimd.collective_compute(
                "AllReduce", OP.add,
                replica_groups=[list(range(N_CORES))],
                ins=[cc_in.opt()], outs=[cc_out.opt()])
            nc.sync.dma_start(cc_res[:], cc_out[:])
        else:
            nc.vector.tensor_copy(cc_res[:], cc_sb[:])

        t0 = singles.tile([1, 1], F32, tag="t0")
        nc.scalar.mul(t0[0:1, :], cc_res[0:1, 0:1], 1.0 / (T * M_TOT))
        t1 = singles.tile([1, 1], F32, tag="t1")
        # l2p already holds -entropy partials, so L2 = +sum/M
        nc.scalar.mul(t1[0:1, :], cc_res[0:1, 1:2], LMBDA / M_TOT)
        out_sb = singles.tile([1, 1], F32, tag="out_sb")
        nc.vector.tensor_add(out_sb[0:1, :], t0[0:1, :], t1[0:1, :])
        nc.sync.dma_start(out_dram.ap(), out_sb[:])

    nc.compile()
    return nc


_NC_CACHE = None


def _get_nc():
    global _NC_CACHE
    if _NC_CACHE is None:
        _NC_CACHE = build_nc()
    return _NC_CACHE


def _run(block_feats: np.ndarray, trace: bool = False):
    nc = _get_nc()
    x = np.asarray(block_feats, dtype=np.float32)
    assert x.shape == (T, N_CORES * COLS), x.shape
    in_maps = [
        {"x": np.ascontiguousarray(x[:, c * COLS:(c + 1) * COLS])}
        for c in range(N_CORES)
    ]
    res = run_bass_kernel_spmd(nc, in_maps, list(range(N_CORES)), trace=trace)
    val = np.float32(res.results[0]["out"][0, 0])
    return val, res


def kernel(block_feats: np.ndarray) -> np.ndarray:
    val, _ = _run(block_feats)
    return np.array(val, dtype=np.float32)


if __name__ == "__main__":
    rng = np.random.default_rng(0)
    xf = rng.standard_normal((T, N_CORES * COLS), dtype=np.float32)
    v = kernel(xf)
    print("kernel out:", v)


# revision 26
# speedup vs baseline: 1.3976x; 1.0967x over previous
"""ClusterLoss (mean-entropy + batch-entropy) Bass kernel for 8 trn2 cores.

Problem: block_feats [T=4096, M*K=64*256] f32.
  x = reshape(T, M, K)
  L1 = mean over (T, M) of entropy(softmax(x, axis=K))
  L2 = -sum_m entropy(softmax(mean_t x)) / M
  out = L1 + L2   (scalar)

Sharding: columns across 8 cores (each core: 8 blocks x all 4096 rows).
Single AllReduce of [1, 2] f32 combines per-core partials.

Per-core plan (rows on partitions, 8 super-tiles of [128, 4, 2048] bf16):
Each (row, block) needs s = sum_k e^x and u = sum_k x*e^x.  The s-accum
runs on DVE tensor_scalar (bf16 4x mode, 127ns/seg).  The u-pass is the
expensive part (no 4x op computes a two-tensor product-reduce), so it is
split per a-group across three methods to balance engines:
  'D': DVE tensor_tensor mult (2x) writes w = x*e in place over e, then
       TS-accum (4x) sums it.
  'P': the TT mult runs on the otherwise-idle Pool/GPSIMD engine
       (1x, 0.42 efficiency) into the e2 tile; DVE only does the accum.
  'F': centered finite difference: ACT computes e+ = exp((1+d/2)x) and
       e- = exp((1-d/2)x) (scale is free on ACT); TS-accum gives
       S+ and S-; then s = (S+ + S-)/2, u = (S+ - S-)/d with O(d^2)
       bias ~1e-5.  Costs a second ACT exp pass but no DVE mult.
Column sums for L2 run on PE (ones-matmul, dependency-gated so the
p-state model gives full clock) into a [4, 512] PSUM tile; the L2
entropy chain runs off the critical path right after the last matmul.
"""

import sys

sys.path.insert(0, "/opt/trn_rl_repo")

import numpy as np

import concourse.bass as bass
import concourse.bacc as bacc
import concourse.tile as tile
from concourse import mybir
from concourse.bass_utils import run_bass_kernel_spmd

F32 = mybir.dt.float32
BF16 = mybir.dt.bfloat16
AF = mybir.ActivationFunctionType
OP = mybir.AluOpType

# Problem constants
T = 4096            # rows (batch)
M_TOT = 64          # blocks
K = 256             # features per block
N_CORES = 8
COLS = (M_TOT * K) // N_CORES   # 2048 columns per core
M_LOC = COLS // K               # 8 blocks per core
P = 128                         # partitions
A = 4                           # row-groups packed per super-tile
ROWS_PER_TILE = P * A           # 512
NT = T // ROWS_PER_TILE         # 8 super-tiles

LMBDA = 1.0

# --- tuning knobs -----------------------------------------------------------
# Per-a-group u-pass method, indexed [it][a].  'P' = Pool TT, 'D' = DVE TT,
# 'F' = finite difference (second scaled exp on ACT).
AG_METHOD = [
    "PPDD",   # tile 0
    "PPDD",   # tile 1
    "PPDD",   # tile 2
    "PPDD",   # tile 3
    "PPFF",   # tile 4
    "PPFF",   # tile 5
    "PPFF",   # tile 6
    "FFFF",   # tile 7
]
DELTA = 2.0 ** -6
BUFS = 4             # rotation depth for x/e pools
BUFS2 = 3            # rotation depth for e2 pool (Pool-w / FD e-)
# Pool-tile deferred u-accum flush points: pool tile -> tile at whose DVE
# block the u-TS is emitted (late enough that the Pool TT chain has surely
# produced w; early enough not to pile into the tail).
POOL_FLUSH_AT = {0: 1, 1: 2, 2: 3, 3: 4, 4: 5, 5: 6, 6: 7}
USE_COLLECTIVE = True


def _absorb_deps(eng, dst_col, dep_insts):
    """Absorb cross-engine waits on `eng`'s queue before a wait-slot-limited
    instruction (e.g. SWDGE pseudo-DMA): one tiny input-free write per
    dependency, each carrying a single sem wait, advancing the engine's
    observed vector clock."""
    from concourse.tile_rust import add_dep_helper

    for j, di in enumerate(dep_insts):
        if hasattr(eng, "memset"):
            c = eng.memset(dst_col[:, j:j + 1], 0.0)
        else:
            c = eng.memzero(dst_col[:, j:j + 1])  # ScalarE
        add_dep_helper(c.ins, di.ins, reason="absorb wait for slot-limited op")


def _absorb(eng, dst_col, src_aps):
    """Absorb cross-engine waits: tiny copies that read the freshly produced
    tiles. Each copy carries one sem wait; once the engine has waited, its
    observed vector clock covers the tick, so following single-wait-slot
    instructions need no cross-engine waits."""
    for j, src in enumerate(src_aps):
        eng.tensor_copy(dst_col[:, j:j + 1], src)


def _runs(methods):
    """Contiguous same-method runs of an AG_METHOD string: [(m, a0, a1))."""
    out = []
    a = 0
    while a < len(methods):
        b = a
        while b < len(methods) and methods[b] == methods[a]:
            b += 1
        out.append((methods[a], a, b))
        a = b
    return out


def build_nc(reps: int = 1):
    assert reps == 1, "only reps=1 supported"
    nc = bacc.Bacc("TRN2", target_bir_lowering=False, debug=False,
                   num_devices=N_CORES)
    x_dram = nc.dram_tensor("x", [T, COLS], F32, kind="ExternalInput")
    out_dram = nc.dram_tensor("out", [1, 1], F32, kind="ExternalOutput")

    from contextlib import ExitStack

    sp = 1.0 + DELTA / 2.0
    sm = 1.0 - DELTA / 2.0

    with tile.TileContext(nc) as tc, ExitStack() as ctx:
        loads = ctx.enter_context(tc.tile_pool(name="loads", bufs=BUFS))
        es = ctx.enter_context(tc.tile_pool(name="es", bufs=BUFS))
        e2s = ctx.enter_context(tc.tile_pool(name="e2s", bufs=BUFS2))
        junks = ctx.enter_context(tc.tile_pool(name="junks", bufs=2))
        singles = ctx.enter_context(tc.tile_pool(name="singles", bufs=1))
        psum = ctx.enter_context(tc.tile_pool(name="psum", bufs=1, space="PSUM"))
        dram = ctx.enter_context(tc.tile_pool(name="dram", bufs=1, space="DRAM"))

        # persistent tiles
        ones_b = singles.tile([P, 1], BF16, tag="ones_b")      # matmul lhsT
        nc.vector.memset(ones_b, 1.0)
        ones_f32 = singles.tile([P, 1], F32, tag="ones_f32")
        nc.vector.memset(ones_f32, 1.0)
        s_sb = singles.tile([P, NT * A * M_LOC], F32, tag="s_sb")
        u_sb = singles.tile([P, NT * A * M_LOC], F32, tag="u_sb")
        fd_tmp = singles.tile([P, A * M_LOC], F32, tag="fd_tmp")
        # wait-absorber targets (disjoint columns per use)
        ab_v = singles.tile([P, 4 * NT + 8], F32, tag="ab_v")
        ab_dma = singles.tile([P, 6 * NT], F32, tag="ab_dma")
        ab_act = singles.tile([P, 2 * NT + 4], F32, tag="ab_act")
        ab_e2 = singles.tile([P, NT], F32, tag="ab_e2")
        ab_g = singles.tile([P, 2 * NT + 4], F32, tag="ab_g")
        ab_act2 = singles.tile([P, 2 * NT], F32, tag="ab_act2")
        ab_t = singles.tile([1, 4], F32, tag="ab_t")

        # Activation-table warmup off the critical path.  Only one table SET
        # is resident at a time (each Exp<->Ln switch costs a 1283ns load),
        # so warm Ln first, then Exp: the Exp set is resident for the whole
        # exp stream, and the single switch back to Ln happens in the tail.
        warm = singles.tile([1, 2], F32, tag="warm")
        nc.scalar.activation(warm[0:1, 1:2], ones_f32[0:1, 0:1], AF.Ln)
        nc.scalar.activation(warm[0:1, 0:1], ones_f32[0:1, 0:1], AF.Exp)

        # colsum accumulator in PSUM: [1, 2048] f32 (4 banks on partition 0;
        # matmul chunks of 512 stay within a bank)
        ps_cs = psum.tile([1, COLS], F32, tag="ps_cs")

        x_view = x_dram.ap().rearrange("(n a p) c -> n p a c", p=P, a=A)

        hist = {t: {} for t in range(NT)}
        pend_pool = []      # deferred Pool-tile u-TS work: (it, a0, a1)

        def issue_dma(t):
            """SWDGE bf16 cast-load for tile t (split for tile 0)."""
            x_t = loads.tile([P, A, COLS], BF16, tag="x_t", name=f"x{t}")
            hist[t]["x"] = x_t
            if t == 0:
                h = nc.gpsimd.dma_start(out=x_t[:, 0:2], in_=x_view[0][:, 0:2])
                hist[t]["dma"] = h
                h2 = nc.gpsimd.dma_start(out=x_t[:, 2:4], in_=x_view[0][:, 2:4])
                hist[t]["dma2"] = h2
            else:
                h = nc.gpsimd.dma_start(out=x_t[:], in_=x_view[t])
                hist[t]["dma"] = h

        def issue_mms(t):
            """Colsum ones-matmuls for tile t (gated on its DMA)."""
            x_t = hist[t]["x"]
            for a in range(A):
                for c in range(COLS // 512):
                    mm_h = nc.tensor.matmul(
                        ps_cs[0:1, c * 512:(c + 1) * 512],
                        ones_b[:],
                        x_t[:, a, c * 512:(c + 1) * 512],
                        start=(t == 0 and a == 0),
                        stop=(t == NT - 1 and a == A - 1),
                    )
                    hist[t]["mm"] = mm_h

        def flush_pool_u(now_it):
            """Emit deferred u-accums for Pool tiles whose TT is surely done."""
            while pend_pool and POOL_FLUSH_AT.get(pend_pool[0][0], 99) <= now_it:
                pit, a0, a1 = pend_pool.pop(0)
                w_t = hist[pit]["e2"]
                # absorb the Pool-TT-done wait on the DVE queue
                _absorb(nc.vector, ab_v[:, 4 * pit + 2:4 * pit + 3],
                        [w_t[:, a0, 0:1]])
                h = None
                for a in range(a0, a1):
                    for m in range(M_LOC):
                        idx = (pit * A + a) * M_LOC + m
                        sl = (slice(None), a, slice(m * K, (m + 1) * K))
                        h = nc.vector.tensor_scalar(
                            out=w_t[sl], in0=w_t[sl], scalar1=1.0,
                            scalar2=None, op0=OP.mult, op1=OP.add,
                            accum_out=u_sb[:, idx:idx + 1])
                hist[pit]["uTS"] = h

        # L2 chain pieces (emitted mid-loop to keep them off the critical
        # path): part A evacuates+scatters the colsum, part B computes the
        # per-block entropies.
        l2 = {}

        def l2_part_a():
            cs_row = singles.tile([1, COLS], F32, tag="cs_row")
            nc.gpsimd.tensor_copy(cs_row[0:1, :], ps_cs[0:1, :])
            cs_sb = singles.tile([8, K], F32, tag="cs_sb")
            nc.sync.dma_start(
                out=cs_sb[0:8, :],
                in_=cs_row[0:1, :].rearrange("p (m k) -> (p m) k", k=K))
            l2["cs_sb"] = cs_sb

        def l2_part_b():
            cs_sb = l2["cs_sb"]
            ebm = singles.tile([8, K], BF16, tag="ebm")
            nc.scalar.activation(ebm[0:8, :], cs_sb[0:8, :], AF.Exp,
                                 scale=1.0 / T)
            tbm = singles.tile([8, K], BF16, tag="tbm")
            # tbm = cs * ebm  (1/T folded into the entropy STT below)
            _absorb(nc.vector, ab_t[0:1, 1:2], [ebm[0:1, 0:1]])
            nc.vector.tensor_tensor(tbm[0:8, :], cs_sb[0:8, :], ebm[0:8, :],
                                    op=OP.mult)
            s_bm = singles.tile([8, 1], F32, tag="s_bm")
            u_bm = singles.tile([8, 1], F32, tag="u_bm")
            nc.vector.tensor_scalar(
                out=ebm[0:8, :], in0=ebm[0:8, :],
                scalar1=1.0, scalar2=None, op0=OP.mult, op1=OP.add,
                accum_out=s_bm[0:8, :])
            nc.vector.tensor_scalar(
                out=tbm[0:8, :], in0=tbm[0:8, :],
                scalar1=1.0, scalar2=None, op0=OP.mult, op1=OP.add,
                accum_out=u_bm[0:8, :])
            ln_sbm = singles.tile([8, 1], F32, tag="ln_sbm")
            nc.scalar.activation(ln_sbm[0:8, :], s_bm[0:8, :], AF.Ln)
            r_sbm = singles.tile([8, 1], F32, tag="r_sbm")
            nc.vector.reciprocal(r_sbm[0:8, :], s_bm[0:8, :])
            q_bm = singles.tile([8, 1], F32, tag="q_bm")
            nc.vector.tensor_tensor(q_bm[0:8, :], u_bm[0:8, :],
                                    r_sbm[0:8, :], op=OP.mult)
            entbm_junk = singles.tile([8, 1], F32, tag="entbm_junk")
            l2p_col = singles.tile([8, 1], F32, tag="l2p_col")
            _absorb(nc.vector, ab_t[0:1, 0:1], [ln_sbm[0:1, 0:1]])
            nc.vector.scalar_tensor_tensor(
                out=entbm_junk[0:8, :], in0=q_bm[0:8, :], scalar=1.0 / T,
                in1=ln_sbm[0:8, :], op0=OP.mult, op1=OP.subtract,
                accum_out=l2p_col[0:8, :])
            # l2p_col = (q/T - ln s) per partition  == -entropy per block
            ps_l2 = psum.tile([1, 1], F32, tag="ps_l2")
            nc.tensor.matmul(ps_l2[0:1, 0:1], ones_f32[0:8],
                             l2p_col[0:8, :], start=True, stop=True)
            l2["ps_l2"] = ps_l2

        # prime the pipeline: first BUFS loads before any Pool TT
        for t in range(BUFS):
            issue_dma(t)

        for it in range(NT):
            methods = AG_METHOD[it]
            has_p = "P" in methods
            has_f = "F" in methods
            x_t = hist[it]["x"]

            # ---- ACT: exp passes -------------------------------------------
            if it >= 1:
                # e-slot WAR: absorb old e readers on the scalar queue
                pv = hist.get(it - BUFS, {})
                deps = [pv[k] for k in ("sTS", "uTS") if k in pv]
                if deps:
                    _absorb_deps(nc.scalar,
                                 ab_act[:, 2 * it:2 * it + len(deps)], deps)
            _absorb_deps(nc.scalar, ab_act2[:, 2 * it:2 * it + 1],
                         [hist[it]["dma"]])
            if "dma2" in hist[it]:
                _absorb_deps(nc.scalar, ab_act2[:, 2 * it + 1:2 * it + 2],
                             [hist[it]["dma2"]])
            e_t = es.tile([P, A, COLS], BF16, tag="e_t", name=f"e{it}")
            hist[it]["e"] = e_t
            # e2 slot WAR: absorb old slot readers before its writers
            e2_t = None
            if has_p or has_f:
                pv = hist.get(it - BUFS2, {})
                if "e2" in pv and "uTS" in pv:
                    _absorb_deps(nc.scalar, ab_e2[:, it:it + 1], [pv["uTS"]])
                    _absorb_deps(nc.gpsimd, ab_g[:, 2 * it:2 * it + 1],
                                 [pv["uTS"]])
                e2_t = e2s.tile([P, A, COLS], BF16, tag="e2_t",
                                name=f"e2{it}")
                hist[it]["e2"] = e2_t

            # plain exp covers the contiguous P/D prefix in one op; F spans
            # get the scaled pair.  Tile 0 splits its plain exp for pipeline
            # fill; the last tile splits its F pairs per a-group so the final
            # accums chase each exp.
            n_plain = sum(1 for c in methods if c in "PD")
            if it == 0:
                h = nc.scalar.activation(e_t[:, 0:2], x_t[:, 0:2], AF.Exp)
                hist[it]["act_p"] = h
                h = nc.scalar.activation(e_t[:, 2:4], x_t[:, 2:4], AF.Exp)
                hist[it]["act"] = h
            elif it == NT - 1 and methods == "FFFF":
                for a in range(A):
                    h = nc.scalar.activation(e_t[:, a:a + 1], x_t[:, a:a + 1],
                                             AF.Exp, scale=sp)
                    hist[it]["act"] = h
                    h2 = nc.scalar.activation(e2_t[:, a:a + 1],
                                              x_t[:, a:a + 1], AF.Exp,
                                              scale=sm)
                    hist[it]["act2"] = h2
            else:
                if n_plain:
                    h = nc.scalar.activation(e_t[:, 0:n_plain],
                                             x_t[:, 0:n_plain], AF.Exp)
                    hist[it]["act_p"] = h
                    hist[it]["act"] = h
                if n_plain < A:
                    h = nc.scalar.activation(e_t[:, n_plain:A],
                                             x_t[:, n_plain:A], AF.Exp,
                                             scale=sp)
                    hist[it]["act"] = h
                    h2 = nc.scalar.activation(e2_t[:, n_plain:A],
                                              x_t[:, n_plain:A], AF.Exp,
                                              scale=sm)
                    hist[it]["act2"] = h2

            # ---- Pool: TT mult for 'P' a-groups ----------------------------
            if has_p:
                for meth, a0, a1 in _runs(methods):
                    if meth != "P":
                        continue
                    _absorb_deps(nc.gpsimd, ab_g[:, 2 * it + 1:2 * it + 2],
                                 [hist[it]["act_p"]])
                    tt_h = nc.gpsimd.tensor_tensor(
                        e2_t[:, a0:a1], x_t[:, a0:a1], e_t[:, a0:a1],
                        op=OP.mult)
                    hist[it]["ptt"] = tt_h
                    pend_pool.append((it, a0, a1))

            # ---- Pool: next SWDGE load (after this tile's TT) --------------
            nt_ = it + BUFS - 1
            if 1 <= it and nt_ < NT:
                pv = hist[it - 1]
                deps = [pv[k] for k in ("act", "mm", "tt", "sTS", "uTS")
                        if k in pv]
                _absorb_deps(nc.gpsimd,
                             ab_dma[:, 6 * it:6 * it + len(deps)], deps)
                issue_dma(nt_)

            # ---- PE: colsum matmuls ----------------------------------------
            if it <= NT - 3:
                issue_mms(it)
            if it == NT - 3:
                issue_mms(NT - 2)
                issue_mms(NT - 1)

            # ---- DVE: accums (and D-tile TT + u-accums) --------------------
            flush_pool_u(it)
            ab_srcs = [e_t[:, 0, 0:1]]
            if "D" in methods:
                ab_srcs.append(x_t[:, 0, 0:1])
            if has_f:
                f_a0 = next(a0 for meth, a0, a1 in _runs(methods)
                            if meth == "F")
                ab_srcs.append(e2_t[:, f_a0, 0:1])
            _absorb(nc.vector, ab_v[:, 4 * it:4 * it + len(ab_srcs)], ab_srcs)

            junk = None
            if has_p:
                junk = junks.tile([P, COLS], BF16, tag="junk", name=f"j{it}")

            for meth, a0, a1 in _runs(methods):
                h = None
                for a in range(a0, a1):
                    for m in range(M_LOC):
                        idx = (it * A + a) * M_LOC + m
                        sl = (slice(None), a, slice(m * K, (m + 1) * K))
                        jl = (slice(None), slice(m * K, (m + 1) * K))
                        # s = sum_k e (for F: S+)
                        out_ap = junk[jl] if meth == "P" else e_t[sl]
                        h = nc.vector.tensor_scalar(
                            out=out_ap, in0=e_t[sl], scalar1=1.0,
                            scalar2=None, op0=OP.mult, op1=OP.add,
                            accum_out=s_sb[:, idx:idx + 1])
                        hist[it]["sTS"] = h
                        if meth == "F":
                            # S- accum into u_sb (normalized below)
                            h = nc.vector.tensor_scalar(
                                out=e2_t[sl], in0=e2_t[sl], scalar1=1.0,
                                scalar2=None, op0=OP.mult, op1=OP.add,
                                accum_out=u_sb[:, idx:idx + 1])
                            hist[it]["uTS"] = h
                if meth == "D":
                    # w = x*e in place over e (2x TT), then accum each seg
                    tt_h = nc.vector.tensor_tensor(
                        e_t[:, a0:a1], x_t[:, a0:a1], e_t[:, a0:a1],
                        op=OP.mult)
                    hist[it]["tt"] = tt_h
                    for a in range(a0, a1):
                        for m in range(M_LOC):
                            idx = (it * A + a) * M_LOC + m
                            sl = (slice(None), a, slice(m * K, (m + 1) * K))
                            h = nc.vector.tensor_scalar(
                                out=e_t[sl], in0=e_t[sl], scalar1=1.0,
                                scalar2=None, op0=OP.mult, op1=OP.add,
                                accum_out=u_sb[:, idx:idx + 1])
                            hist[it]["uTS"] = h

            # FD normalization for this tile's F columns:
            #   s <- (S+ + S-)/2 ; u <- (S+ - S-)/DELTA
            for meth, a0, a1 in _runs(methods):
                if meth != "F":
                    continue
                c0 = (it * A + a0) * M_LOC
                c1 = (it * A + a1) * M_LOC
                n = c1 - c0
                nc.vector.tensor_tensor(
                    fd_tmp[:, 0:n], s_sb[:, c0:c1], u_sb[:, c0:c1],
                    op=OP.subtract)
                nc.vector.tensor_tensor(
                    s_sb[:, c0:c1], s_sb[:, c0:c1], u_sb[:, c0:c1],
                    op=OP.add)
                nc.vector.tensor_scalar(
                    out=s_sb[:, c0:c1], in0=s_sb[:, c0:c1], scalar1=0.5,
                    scalar2=None, op0=OP.mult, op1=OP.add)
                nc.vector.tensor_scalar(
                    out=u_sb[:, c0:c1], in0=fd_tmp[:, 0:n],
                    scalar1=1.0 / DELTA,
                    scalar2=None, op0=OP.mult, op1=OP.add)

            # ---- L2 colsum evacuation, off the critical path ---------------
            if it == NT - 3:
                l2_part_a()

        # flush any remaining deferred Pool u-accums
        flush_pool_u(99)
        # L2 entropy chain: its ebm exp is the last Exp op, so the Ln ops
        # below it and in the L1 tail cost a single table switch.
        l2_part_b()
        ps_l2 = l2["ps_l2"]

        # ---- L1 tail: ent = ln s - u/s over all 256 cols --------------------
        n_col = NT * A * M_LOC  # 256
        ln_s = singles.tile([P, n_col], F32, tag="ln_s")
        nc.scalar.activation(ln_s[:], s_sb[:], AF.Ln)
        rs = singles.tile([P, n_col], F32, tag="rs")
        nc.vector.reciprocal_approx_fast(rs[:], s_sb[:])
        q = singles.tile([P, n_col], F32, tag="q")
        nc.vector.tensor_tensor(q[:], u_sb[:], rs[:], op=OP.mult)
        ent_junk = singles.tile([P, n_col], F32, tag="ent_junk")
        l1p = singles.tile([P, 1], F32, tag="l1p")
        _absorb(nc.vector, ab_v[:, 4 * NT + 1:4 * NT + 2], [ln_s[:, 0:1]])
        nc.vector.scalar_tensor_tensor(
            out=ent_junk[:], in0=ln_s[:], scalar=1.0, in1=q[:],
            op0=OP.mult, op1=OP.subtract, accum_out=l1p[:])
        ps_l1 = psum.tile([1, 1], F32, tag="ps_l1")
        nc.tensor.matmul(ps_l1[0:1, 0:1], ones_f32[:], l1p[:],
                         start=True, stop=True)

        # ---- pack partials, AllReduce, final scalar -------------------------
        cc_sb = singles.tile([1, 2], F32, tag="cc_sb")
        nc.scalar.copy(cc_sb[0:1, 0:1], ps_l1[0:1, 0:1])
        nc.scalar.copy(cc_sb[0:1, 1:2], ps_l2[0:1, 0:1])
        cc_res = singles.tile([1, 2], F32, tag="cc_res")
        if USE_COLLECTIVE:
            cc_in = dram.tile([1, 2], F32, tag="cc_in")
            cc_out = dram.tile([1, 2], F32, tag="cc_out")
            nc.gpsimd.dma_start(cc_in[:], cc_sb[:])
            nc.gpsimd.collective_compute(
                "AllReduce", OP.add,
                replica_groups=[list(range(N_CORES))],
                ins=[cc_in.opt()], outs=[cc_out.opt()])
            nc.sync.dma_start(cc_res[:], cc_out[:])
        else:
            nc.vector.tensor_copy(cc_res[:], cc_sb[:])

        t0 = singles.tile([1, 1], F32, tag="t0")
        nc.scalar.mul(t0[0:1, :], cc_res[0:1, 0:1], 1.0 / (T * M_TOT))
        t1 = singles.tile([1, 1], F32, tag="t1")
        # l2p already holds -entropy partials, so L2 = +sum/M
        nc.scalar.mul(t1[0:1, :], cc_res[0:1, 1:2], LMBDA / M_TOT)
        out_sb = singles.tile([1, 1], F32, tag="out_sb")
        nc.vector.tensor_add(out_sb[0:1, :], t0[0:1, :], t1[0:1, :])
        nc.sync.dma_start(out_dram.ap(), out_sb[:])

    nc.compile()
    return nc


_NC_CACHE = None


def _get_nc():
    global _NC_CACHE
    if _NC_CACHE is None:
        _NC_CACHE = build_nc()
    return _NC_CACHE


def _run(block_feats: np.ndarray, trace: bool = False):
    nc = _get_nc()
    x = np.asarray(block_feats, dtype=np.float32)
    assert x.shape == (T, N_CORES * COLS), x.shape
    in_maps = [
        {"x": np.ascontiguousarray(x[:, c * COLS:(c + 1) * COLS])}
        for c in range(N_CORES)
    ]
    res = run_bass_kernel_spmd(nc, in_maps, list(range(N_CORES)), trace=trace)
    val = np.float32(res.results[0]["out"][0, 0])
    return val, res


def kernel(block_feats: np.ndarray) -> np.ndarray:
    val, _ = _run(block_feats)
    return np.array(val, dtype=np.float32)


if __name__ == "__main__":
    rng = np.random.default_rng(0)
    xf = rng.standard_normal((T, N_CORES * COLS), dtype=np.float32)
    v = kernel(xf)
    print("kernel out:", v)
